# revision 11
# baseline (speedup 1.0000x reference)
"""Trainium2 Bass kernel: single-layer GRU (T=512, B=64, F=128, H=512) + output proj (O=16).

Sharding: data-parallel over batch. B=64 -> 8 cores x 8 sequences each.
Weights replicated; the recurrence is fully local per core.

Per-core layout (everything "hidden-dim on partitions"):
  x_sb    [128(f), T*8(t,b)]            bf16
  w_ih_sb [128(f), 12*128(g')]          bf16   (gate-chunk-permuted columns)
  w_hh_sb [128(k), 4(hc), 12*128(g')]   bf16
  xg      8 tiles [128(g'p), 12(g'c), 64*8(t,b)] bf16  (x-side gates + biases)
  hs_sb   [128(hp), T, 4(hc), 8(b)]     bf16   (hidden history; rhs of the next
                                               step's matmul and phase-3 input)

Device gate-chunk order g' = [r0,r1,z0,z1, r2,r3,z2,z3, n0,n1,n2,n3]: each
"half" of the hidden state (chunks 0-1 / 2-3) has its r/z/n slices contiguous,
so the elementwise GRU update runs per-half and pipelines against the PE.

Per-step structure (the critical path is the chain sigmoid->stt->add->tanh->
mul->add; everything else is arranged to stay off it):
  - ps_rz[half] accumulation group OPENS with an identity matmul that injects
    the precomputed x-side pre-activations (incl. biases) straight into PSUM
    (start=True, no dependency on h -> scheduled early); the 16 W_hh matmuls
    then accumulate on top. Sigmoid reads PSUM directly - no xg add on the
    chain.
  - 1-z is sigmoid(-z_pre) on the scalar engine, z*h_prev on the Pool engine:
    both off-chain, so the post-tanh tail is just mul+add.
  - the final add writes h as bf16 directly into hs_sb (no f32 copy).
  - PE order: [I0,I1 | all hc0/1 matmuls | hc2/3 of half0 | hc2/3 of half1] so
    matmuls needing h(t-1) half1 sit late (its chain finishes during the step)
    and ps/half0 completes as early as possible.

build_nc(reps=N) wraps the whole body (xg precompute + recurrence + output
proj) in a tc.For_i hardware loop: N back-to-back executions in one launch,
used by the test harness to measure true HW time under the ~75ms axon
dispatch overhead (output is written identically by every iteration).
"""

import numpy as np
import ml_dtypes
from contextlib import ExitStack

import concourse.bass as bass
import concourse.tile as tile
from concourse import bacc, mybir
from concourse.bass import ds, ts
from concourse.bass_utils import run_bass_kernel_spmd

T, B, F, H, O = 512, 64, 128, 512, 16
N_CORES = 8
BL = B // N_CORES          # 8 sequences per core
GC = (3 * H) // 128        # 12 gate chunks
HC = H // 128              # 4 hidden chunks
TCH = 8                    # xg is staged in 8 chunks of 64 timesteps
TC = T // TCH              # 64
# device gate-chunk order (indices into original [r0..r3, z0..z3, n0..n3])
PERM_BLOCKS = [0, 1, 4, 5, 2, 3, 6, 7, 8, 9, 10, 11]

F32 = mybir.dt.float32
BF16 = mybir.dt.bfloat16
FP8 = mybir.dt.float8e4
BF_NP = ml_dtypes.bfloat16
FP8_NP = ml_dtypes.float8_e4m3
ADD = mybir.AluOpType.add
MULT = mybir.AluOpType.mult

# NOTE: an fp8e4m3 W_hh variant (x32 scale, descale via activation scale=)
# was measured SLOWER on hw (2.29ms vs 1.84ms amortized): at moving-free-dim 8
# the fp8 path loses FWL (the documented small-FD fp8 trap), so weights stay
# bf16. WSCALE kept at 1 so the scale plumbing remains testable.
WSCALE = 1.0
INV_WSCALE = 1.0 / WSCALE
# COLTILE>1 would split each 128-col W_hh load into col-group (M=128/n)
# matmuls hoping per-subarray XBUSes overlap the loads; blocked in practice:
# AP base partitions are restricted to {0,32,64} (no 96), and 2x tiling can
# only tie FWL's 2x, so the full-width FWL load is kept.
COLTILE = 1


def build_nc(t_steps: int = T, reps: int = 1):
    """Build + compile the per-core Bass program (SPMD: same program, 8 cores)."""
    FT = mybir.ActivationFunctionType
    nc = bacc.Bacc("TRN2", target_bir_lowering=False, debug=False,
                   num_devices=N_CORES)

    x_in = nc.dram_tensor("x", [128, T * BL], BF16, kind="ExternalInput")
    whh_in = nc.dram_tensor("w_hh_t", [HC, 128, GC * 128], BF16, kind="ExternalInput")
    wih_in = nc.dram_tensor("w_ih_t", [128, GC * 128], BF16, kind="ExternalInput")
    bias_in = nc.dram_tensor("biasg", [128, GC], F32, kind="ExternalInput")
    bhn_in = nc.dram_tensor("bhn", [128, HC], F32, kind="ExternalInput")
    wout_in = nc.dram_tensor("w_out_t", [HC, 128, O], BF16, kind="ExternalInput")
    bout_in = nc.dram_tensor("b_out_p", [O, 1], F32, kind="ExternalInput")
    ident_in = nc.dram_tensor("ident", [128, 128], BF16, kind="ExternalInput")
    y_out = nc.dram_tensor("y", [O, T * BL], F32, kind="ExternalOutput")

    with tile.TileContext(nc) as tc, ExitStack() as ctx:
        const = ctx.enter_context(tc.tile_pool(name="const", bufs=1))
        psum = ctx.enter_context(tc.tile_pool(name="psum", bufs=2, space="PSUM"))
        work = ctx.enter_context(tc.tile_pool(name="work", bufs=2))

        # ---- constants / inputs to SBUF
        x_sb = const.tile([128, T * BL], BF16)
        nc.sync.dma_start(x_sb[:], x_in.ap()[:])
        whh_sb = const.tile([128, HC, GC * 128], BF16)
        for hc in range(HC):
            nc.sync.dma_start(whh_sb[:, hc, :], whh_in.ap()[hc])
        wih_sb = const.tile([128, GC * 128], BF16)
        nc.sync.dma_start(wih_sb[:], wih_in.ap()[:])
        bias_sb = const.tile([128, GC], F32)
        nc.sync.dma_start(bias_sb[:], bias_in.ap()[:])
        bhn_sb = const.tile([128, HC], F32)
        nc.sync.dma_start(bhn_sb[:], bhn_in.ap()[:])
        wout_sb = const.tile([128, HC, O], BF16)
        for hc in range(HC):
            nc.sync.dma_start(wout_sb[:, hc, :], wout_in.ap()[hc])
        bout_sb = const.tile([O, 1], F32)
        nc.sync.dma_start(bout_sb[:], bout_in.ap()[:])
        ident_sb = const.tile([128, 128], BF16)
        nc.sync.dma_start(ident_sb[:], ident_in.ap()[:])

        hs_sb = const.tile([128, T, HC, BL], BF16)
        h0_bf = const.tile([128, HC, BL], BF16)
        nc.vector.memset(h0_bf[:], 0)
        xg_tiles = [const.tile([128, GC, TC * BL], BF16, name=f"xg{i}")
                    for i in range(TCH)]

        def body():
            # ---- phase 1: xg[g', (t,b)] = w_ih' . x + biasg (permuted order;
            # rz chunks carry b_ih+b_hh, n chunks carry b_ih only)
            for c in range(TCH):
                for g in range(GC):
                    ps = psum.tile([128, TC * BL], F32, tag=f"p{g % 4}")
                    nc.tensor.matmul(ps[:], wih_sb[:, ts(g, 128)],
                                     x_sb[:, ts(c, TC * BL)], start=True, stop=True)
                    dst = xg_tiles[c][:, g, :]
                    if g % 2 == 0:
                        nc.scalar.activation(dst, ps[:], FT.Identity,
                                             bias=bias_sb[:, g:g + 1], scale=1.0)
                    else:
                        nc.vector.tensor_scalar_add(dst, ps[:], bias_sb[:, g:g + 1])

            # ---- phase 2: the recurrence
            for t in range(t_steps):
                c, tt = divmod(t, TC)
                xg = xg_tiles[c]
                tb = ds(tt * BL, BL)
                rhs = h0_bf if t == 0 else hs_sb[:, t - 1, :, :]

                ps_rz = [psum.tile([128, 4, BL], F32, tag="p0", name="ps_rz0"),
                         psum.tile([128, 4, BL], F32, tag="p2", name="ps_rz1")]
                ps_n = [psum.tile([128, 2, BL], F32, tag="p1", name="ps_n0"),
                        psum.tile([128, 2, BL], F32, tag="p3", name="ps_n1")]

                # PE: identity matmuls open the rz groups with the x-side
                # pre-activations (no h dependency -> issue early).
                for half in (0, 1):
                    nc.tensor.matmul(ps_rz[half][:], ident_sb[:],
                                     xg[:, 4 * half:4 * half + 4, tb],
                                     start=True, stop=False, skip_group_check=True)
                # PE: W_hh accumulation. Emission order = PE queue order, and
                # it is tuned so BOTH halves' elementwise chains (~1.2us) fit
                # inside their PE-overlap windows in steady state:
                #   [A] half0 x hc01          (needs h(t-1) half0)
                #   [B] 6 mms of half1 x hc01 (filler before h1 is needed)
                #   [C] half0 x hc23          (first h(t-1)-half1 use at
                #                              ~40% of the step; ps/half0
                #                              closes at ~64%, leaving
                #                              ~1.2us before h0(t) is needed
                #                              by the next step's [A])
                #   [D] rest of half1 x hc01
                #   [E] half1 x hc23          (ps/half1 closes at step end)
                MQ = 128 // COLTILE

                def rz_mm(half, j, hc, stop=False):
                    g = 4 * half + j
                    for q in range(COLTILE):
                        nc.tensor.matmul(
                            ps_rz[half][MQ * q:MQ * (q + 1), j, :],
                            whh_sb[:, hc, ds(g * 128 + MQ * q, MQ)],
                            rhs[:, hc, :],
                            start=False, stop=(stop and q == COLTILE - 1),
                            skip_group_check=True)

                def n_mm(half, j, hc):
                    g = 8 + 2 * half + j
                    for q in range(COLTILE):
                        nc.tensor.matmul(
                            ps_n[half][MQ * q:MQ * (q + 1), j, :],
                            whh_sb[:, hc, ds(g * 128 + MQ * q, MQ)],
                            rhs[:, hc, :],
                            start=(hc == 0 and j == 0 and q == 0),
                            stop=(hc == 3 and j == 1 and q == COLTILE - 1),
                            skip_group_check=True)

                for j in range(4):                      # [A]
                    for hc in (0, 1):
                        rz_mm(0, j, hc)
                for j in range(2):
                    for hc in (0, 1):
                        n_mm(0, j, hc)
                for j in range(3):                      # [B]
                    for hc in (0, 1):
                        rz_mm(1, j, hc)
                for j in range(4):                      # [C]
                    for hc in (2, 3):
                        rz_mm(0, j, hc, stop=(hc == 3 and j == 3))
                for j in range(2):
                    for hc in (2, 3):
                        n_mm(0, j, hc)
                for hc in (0, 1):                       # [D]
                    rz_mm(1, 3, hc)
                for j in range(2):
                    for hc in (0, 1):
                        n_mm(1, j, hc)
                for j in range(4):                      # [E]
                    for hc in (2, 3):
                        rz_mm(1, j, hc, stop=(hc == 3 and j == 3))
                for j in range(2):
                    for hc in (2, 3):
                        n_mm(1, j, hc)

                # elementwise per half: critical chain is
                # sigmoid -> stt -> add -> tanh -> mul -> add(bf16 to hs_sb).
                # PSUM values are x WSCALE; activations descale via scale=.
                for half in (0, 1):
                    hh = ds(2 * half, 2)
                    rs = work.tile([128, 4, BL], F32, tag=f"rs{half}")
                    nc.scalar.activation(rs[:], ps_rz[half][:], FT.Sigmoid,
                                         scale=INV_WSCALE)
                    # off-chain: 1-z = sigmoid(-z_pre); zh = z*h_prev (Pool)
                    b1z = work.tile([128, 2, BL], F32, tag=f"bz{half}")
                    nc.scalar.activation(b1z[:], ps_rz[half][:, 2:4, :],
                                         FT.Sigmoid, scale=-INV_WSCALE)
                    zh = work.tile([128, 2, BL], F32, tag=f"zh{half}")
                    nc.gpsimd.tensor_mul(zh[:], rs[:, 2:4, :], rhs[:, hh, :])
                    # n-gate: (hn + b_hn) * r + xn (all x WSCALE), tanh descales
                    nm = work.tile([128, 2, BL], F32, tag=f"nm{half}")
                    for j in range(2):
                        k = 2 * half + j
                        nc.vector.scalar_tensor_tensor(
                            nm[:, j, :], ps_n[half][:, j, :],
                            bhn_sb[:, k:k + 1], rs[:, j, :],
                            op0=ADD, op1=MULT)
                    np_ = work.tile([128, 2, BL], F32, tag=f"np{half}")
                    nc.vector.tensor_add(np_[:], nm[:],
                                         xg[:, 8 + 2 * half:10 + 2 * half, tb])
                    nt = work.tile([128, 2, BL], F32, tag=f"nt{half}")
                    nc.scalar.activation(nt[:], np_[:], FT.Tanh,
                                         scale=INV_WSCALE)
                    # tail on Pool: consecutive same-engine ops, no sem hop
                    u = work.tile([128, 2, BL], F32, tag=f"u{half}")
                    nc.gpsimd.tensor_mul(u[:], nt[:], b1z[:])
                    nc.gpsimd.tensor_add(hs_sb[:, t, hh, :], u[:], zh[:])

            # ---- phase 3: y = w_out . h_t + b_out
            for c in range(TCH):
                ps = psum.tile([O, TC * BL], F32, tag="p0")
                for hc in range(HC):
                    nc.tensor.matmul(ps[:], wout_sb[:, hc, :],
                                     hs_sb[:, ts(c, TC), hc, :],
                                     start=(hc == 0), stop=(hc == 3))
                yt = work.tile([O, TC * BL], F32, tag="yt")
                nc.scalar.activation(yt[:], ps[:], FT.Identity, bias=bout_sb[:],
                                     scale=1.0)
                nc.sync.dma_start(y_out.ap()[:, ts(c, TC * BL)], yt[:])

        if reps == 1:
            body()
        else:
            with tc.For_i(0, reps):
                body()

    nc.compile()
    return nc


def prep_inputs(x_rnn, w_ih, w_hh, b_ih, b_hh, w_out, b_out):
    """Host-side shard + relayout. Returns per-core in_maps."""
    x_rnn = np.asarray(x_rnn, np.float32)
    w_ih = np.asarray(w_ih, np.float32)
    w_hh = np.asarray(w_hh, np.float32)
    b_ih = np.asarray(b_ih, np.float32)
    b_hh = np.asarray(b_hh, np.float32)
    w_out = np.asarray(w_out, np.float32)
    b_out = np.asarray(b_out, np.float32)

    rows = np.concatenate([np.arange(b * 128, (b + 1) * 128) for b in PERM_BLOCKS])
    w_ih_p = w_ih[rows]                       # (1536, 128), permuted gate order
    w_hh_p = w_hh[rows]                       # (1536, 512)
    # r/z gates: fold both biases into xg. n gates: only b_ih (b_hn lives
    # inside the r* product and is applied during the recurrence).
    bsum = (b_ih + b_hh)[rows]
    bsum[8 * 128:] = b_ih[rows][8 * 128:]
    # the whole gate pre-activation pipeline runs x WSCALE (fp8 weight scale)
    biasg = bsum.reshape(GC, 128).T.copy() * WSCALE             # (128, GC) f32
    bhn = b_hh[2 * H:].reshape(HC, 128).T.copy() * WSCALE       # (128, HC) f32

    w_ih_t = np.ascontiguousarray(w_ih_p.T * WSCALE).astype(BF_NP)  # (128, 1536)
    w_hh_t = np.ascontiguousarray(
        w_hh_p.T.reshape(HC, 128, GC * 128) * WSCALE).astype(BF_NP)
    w_out_t = np.ascontiguousarray(w_out.T.reshape(HC, 128, O)).astype(BF_NP)
    b_out_p = b_out.reshape(O, 1).astype(np.float32)
    ident = np.eye(128, dtype=BF_NP)

    in_maps = []
    for c in range(N_CORES):
        xc = x_rnn[:, c * BL:(c + 1) * BL, :]             # (T, 8, 128)
        x_t = np.ascontiguousarray(xc.transpose(2, 0, 1).reshape(128, T * BL))
        in_maps.append({
            "x": x_t.astype(BF_NP),
            "w_hh_t": w_hh_t, "w_ih_t": w_ih_t, "biasg": biasg.astype(np.float32),
            "bhn": bhn.astype(np.float32),
            "w_out_t": w_out_t, "b_out_p": b_out_p, "ident": ident,
        })
    return in_maps


def assemble_output(results):
    """results: list of per-core {"y": (O, T*BL)} -> full (T, B, O) f32."""
    ys = []
    for c in range(N_CORES):
        yc = np.asarray(results[c]["y"], np.float32)
        ys.append(yc.reshape(O, T, BL).transpose(1, 2, 0))
    return np.concatenate(ys, axis=1)


_NC_CACHE = {}


def get_nc(t_steps: int = T, reps: int = 1):
    if (t_steps, reps) not in _NC_CACHE:
        _NC_CACHE[(t_steps, reps)] = build_nc(t_steps, reps)
    return _NC_CACHE[(t_steps, reps)]


def kernel(**inputs) -> np.ndarray:
    nc = get_nc()
    in_maps = prep_inputs(**inputs)
    res = run_bass_kernel_spmd(nc, in_maps, list(range(N_CORES)))
    return assemble_output(res.results)
